# revision 64
# baseline (speedup 1.0000x reference)
"""Ragged segment self-attention (AttentionHiddenNet) on 8 Trainium2 cores.

Fixed problem instance: h_states [1, 163840, 64] fp32, 4096 segments whose
lengths cycle through [16, 24, 32, 40, 48, 56, 64, 40] (320 tokens / cycle).
Per segment s: ctx_s = softmax(H_s @ H_s^T, axis=-1) @ H_s.

Sharding: 512 consecutive segments (= 64 cycles = 20480 tokens, contiguous
rows) per core; no cross-core communication.

Per-core algorithm: per 320-token cycle, segments pack into 3 groups
(112/104/104 tokens).  The segment mask folds into the score matmul by
augmenting the hidden dim with 8 one-hot rows of value 30 (one per segment
of the cycle): saug[q,k] = h_q.h_k + 900*same_segment(q,k).  Then
    S~ = Taug^T @ Taug       (PE fp16, K and M zero/spill-padded to 128 so
                              fast-weight-load fires: ~62ns/MM)
    U  = exp(S~ - 1000)      (ACT, one [112,320] op per cycle, bf16 out)
         off-segment exp underflows to exactly 0 = the mask; this also makes
         every padded row/col of U exactly 0, so K/M padding is harmless.
    C|Z = U^T @ [H_g | 1]    (PE bf16, U symmetric, M padded to 128, K=112:
                              ~68ns/MM)
    out = C * (1/Z)          (DVE, batched over 2 cycles: one reciprocal
                              [112,2,3,1] + one broadcast multiply, bf16 out
                              upconverted on host)
Spill/stale regions from the M=128 padding only ever reach output rows
>= L of each group, which the host-side unpack discards.
I/O: host pre-packs token-major tensors so every DMA moves >=6KB per
partition.  ta (fp16 scores operand) on the sync HWDGE ring, h_t (bf16 ctx
operand) on the scalar HWDGE ring, outputs on gpsimd SWDGE (keeps DMA
triggers off the busy ACT queue).  Graduated chunk sizes (6/10/16/16/16
cycles) with 3 rotating ta buffers and 1-2 chunk prefetch distance hide
the transfer ramp.  Steady state is ACT-bound at ~410ns per 320-token
cycle; ~7us NEFF preamble and ~11us semaphore-reset teardown are fixed
framework overhead.
"""

import numpy as np

H_DIM = 64
NUM_SEQS = 4096
LEN_PATTERN = [16, 24, 32, 40, 48, 56, 64, 40]
N_TOTAL = 163840
N_CORES = 8
SEGS_PER_CORE = NUM_SEQS // N_CORES          # 512
CYCLE_TOKS = sum(LEN_PATTERN)                # 320
CYCLES_PER_CORE = SEGS_PER_CORE // len(LEN_PATTERN)   # 64
TOKS_PER_CORE = CYCLES_PER_CORE * CYCLE_TOKS          # 20480

GROUP_TYPES = [(0, 112), (112, 104), (216, 104)]
SEG_STARTS = [0, 16, 40, 72, 112, 160, 216, 280]
IND_VAL = 30.0        # sqrt(900): on-segment score boost +900
BIAS = -1000.0        # exp(s + 900 - 1000) on-seg, exp(s - 1000) -> 0 off-seg

# uneven chunks: small first chunk so compute starts early
CHUNKS = [(0, 6), (6, 10), (16, 16), (32, 16), (48, 16)]  # (start_cyc, n_cyc)
MAX_CYC = 16
N_CHUNKS = len(CHUNKS)

_CACHE = {}


def _expected_sse():
    lens = np.tile(np.array(LEN_PATTERN, dtype=np.int64), NUM_SEQS // len(LEN_PATTERN))
    ends = np.cumsum(lens)
    starts = np.concatenate([[0], ends[:-1]])
    return np.stack([starts, ends], axis=1)


def _build_bass():
    import concourse.bass as bass
    import concourse.bacc as bacc
    import concourse.tile as tile
    from concourse import mybir
    from contextlib import ExitStack

    f32 = mybir.dt.float32
    f16 = mybir.dt.float16
    bf16 = mybir.dt.bfloat16

    nc = bacc.Bacc("TRN2")
    # token-major padded groups: [128, 64 cyc, 3 types, 66]; rows >= L zero
    h_d = nc.dram_tensor(
        "h", [128, CYCLES_PER_CORE, 3, H_DIM + 2], bf16, kind="ExternalInput"
    )
    # augmented transpose, host-built: rows 0-63 h^T, 64-71 segment
    # indicators (value 30 -> +900 on-segment), 72-127 zeros (K=128 pad)
    ht_d = nc.dram_tensor("ht", [128, TOKS_PER_CORE], f16, kind="ExternalInput")
    out_d = nc.dram_tensor(
        "out", [112, CYCLES_PER_CORE, 3, H_DIM], bf16, kind="ExternalOutput"
    )

    with tile.TileContext(nc) as tc, ExitStack() as ctx:
        singles = ctx.enter_context(tc.tile_pool(name="singles", bufs=1))
        iopool = ctx.enter_context(tc.tile_pool(name="iopool", bufs=3))
        upool = ctx.enter_context(tc.tile_pool(name="upool", bufs=4))
        zpool = ctx.enter_context(tc.tile_pool(name="zpool", bufs=4))
        ps_s = ctx.enter_context(tc.tile_pool(name="ps_s", bufs=4, space="PSUM"))
        ps_c = ctx.enter_context(tc.tile_pool(name="ps_c", bufs=3, space="PSUM"))

        bias_t = singles.tile([128, 1], f32, tag="bias")
        nc.gpsimd.memset(bias_t[:, :], BIAS)
        # dummy exp to pull ACT_TABLE_LOAD into the preamble
        warm_t = singles.tile([128, 1], f32, tag="warm")
        nc.scalar.activation(
            warm_t[:, :], bias_t[:, :], mybir.ActivationFunctionType.Exp
        )

        # rotating augmented-transpose tiles [128, MAX_CYC*320]
        ta0 = singles.tile([128, MAX_CYC * CYCLE_TOKS], f16, tag="ta0")
        ta1 = singles.tile([128, MAX_CYC * CYCLE_TOKS], f16, tag="ta1")
        ta2 = singles.tile([128, MAX_CYC * CYCLE_TOKS], f16, tag="ta2")
        ta_bufs = [ta0, ta1, ta2]

        def ta_dma(ck):
            cyc0, n = CHUNKS[ck]
            nc.sync.dma_start(
                ta_bufs[ck % 3][:, 0 : n * CYCLE_TOKS],
                bass.AP(
                    ht_d,
                    cyc0 * CYCLE_TOKS,
                    [[TOKS_PER_CORE, 128], [1, n * CYCLE_TOKS]],
                ),
            )

        def ht_dma(ck):
            cyc0, n = CHUNKS[ck]
            ht_tile = iopool.tile([128, MAX_CYC, 3, H_DIM + 2], bf16, tag="h")
            src = bass.AP(
                h_d,
                cyc0 * 3 * (H_DIM + 2),
                [
                    [CYCLES_PER_CORE * 3 * (H_DIM + 2), 128],
                    [3 * (H_DIM + 2), n],
                    [H_DIM + 2, 3],
                    [1, H_DIM + 2],
                ],
            )
            nc.scalar.dma_start(ht_tile[:, 0:n, :, :], src)
            return ht_tile

        ta_dma(0)
        htq = [ht_dma(0), ht_dma(1)]
        ta_dma(1)
        for ck in range(N_CHUNKS):
            _, n_cyc = CHUNKS[ck]
            chunk_toks = n_cyc * CYCLE_TOKS
            ta = ta_bufs[ck % 3]
            h_t = htq[ck]

            o_all = iopool.tile([112, MAX_CYC, 3, H_DIM], bf16, tag="o")
            for cyc in range(n_cyc):
                if cyc == 2 and ck >= 1 and ck + 1 < N_CHUNKS:
                    ta_dma(ck + 1)
                if cyc == max(0, n_cyc - 8) and ck + 2 < N_CHUNKS:
                    htq.append(ht_dma(ck + 2))
                ktok = cyc * CYCLE_TOKS
                s_ps = ps_s.tile([128, CYCLE_TOKS], f32, tag="s")
                for t, (off, L) in enumerate(GROUP_TYPES):
                    a = ktok + off
                    # M pads to 128 with next-group tokens (their U entries
                    # are exactly 0); the last group of the chunk has no
                    # spill room, so it keeps M=L
                    m = 128 if a + 128 <= chunk_toks else L
                    nc.tensor.matmul(
                        s_ps[0:m, off : off + L],
                        ta[0:128, a : a + m],
                        ta[0:128, a : a + L],
                        start=True,
                        stop=True,
                    )

                u = upool.tile([112, CYCLE_TOKS + 32], bf16, tag="u")
                nc.scalar.activation(
                    u[0:112, 0:CYCLE_TOKS],
                    s_ps[0:112, :],
                    mybir.ActivationFunctionType.Exp,
                    bias=bias_t[0:112, :],
                )

                if cyc % 2 == 0:
                    c_ps = ps_c.tile([128, 2, 3, H_DIM + 2], f32, tag="c")
                for t, (off, L) in enumerate(GROUP_TYPES):
                    # contraction over keys: rows L..112 of u are exact 0
                    # except for the unpadded last group, which must stop at L
                    k = 112 if ktok + off + 128 <= chunk_toks else L
                    nc.tensor.matmul(
                        c_ps[0:128, cyc % 2, t, :],
                        u[0:k, off : off + 128],
                        h_t[0:k, cyc, t, :],
                        start=True,
                        stop=True,
                    )
                if cyc % 2 == 1:
                    r = zpool.tile([112, 2, 3, 1], f32, tag="r")
                    nc.vector.reciprocal(
                        r[0:112, :, :, :], c_ps[0:112, :, :, H_DIM : H_DIM + 1]
                    )
                    nc.vector.tensor_tensor(
                        o_all[0:112, cyc - 1 : cyc + 1, :, :],
                        c_ps[0:112, :, :, 0:H_DIM],
                        r[0:112, :, :, :].to_broadcast((112, 2, 3, H_DIM)),
                        mybir.AluOpType.mult,
                    )

            # two out-DMAs per chunk; the last chunk splits unevenly so the
            # final (teardown-blocking) transfer is small
            splits = (
                [(0, 12), (12, 4)] if ck == N_CHUNKS - 1
                else [(0, n_cyc // 2), (n_cyc // 2, n_cyc // 2)]
            )
            # last chunk's outputs go via the (idle) sync HWDGE ring: its
            # ~0.6us completion beats the ~3us SWDGE end-of-kernel drain
            out_eng = nc.sync if ck == N_CHUNKS - 1 else nc.gpsimd
            for c0, cn in splits:
                dst = bass.AP(
                    out_d,
                    (CHUNKS[ck][0] + c0) * 3 * H_DIM,
                    [
                        [CYCLES_PER_CORE * 3 * H_DIM, 112],
                        [3 * H_DIM, cn],
                        [H_DIM, 3],
                        [1, H_DIM],
                    ],
                )
                out_eng.dma_start(dst, o_all[:, c0 : c0 + cn, :, :])

    nc.compile()
    return nc


def _run_numpy(h, sse):
    # generic host fallback (only used if the input does not match the
    # hardcoded segment pattern)
    out = np.empty_like(h)
    for s, e in sse:
        seg = h[s:e]
        sc = seg @ seg.T
        sc -= sc.max(axis=-1, keepdims=True)
        u = np.exp(sc)
        out[s:e] = (u / u.sum(axis=-1, keepdims=True)) @ seg
    return out


def kernel(h_states, seq_start_end):
    h = np.asarray(h_states, dtype=np.float32).reshape(-1, H_DIM)
    sse = np.asarray(seq_start_end).astype(np.int64)

    if h.shape[0] != N_TOTAL or not np.array_equal(sse, _expected_sse()):
        return _run_numpy(h, sse).astype(np.float32)

    from concourse.bass_utils import run_bass_kernel_spmd
    import ml_dtypes

    if "nc" not in _CACHE:
        _CACHE["nc"] = _build_bass()
    nc = _CACHE["nc"]

    ind = np.zeros((8, CYCLE_TOKS), np.float16)
    for j in range(8):
        ind[j, SEG_STARTS[j] : SEG_STARTS[j] + LEN_PATTERN[j]] = IND_VAL
    ind = np.tile(ind, (1, CYCLES_PER_CORE))

    in_maps = []
    for c in range(N_CORES):
        slab = h[c * TOKS_PER_CORE : (c + 1) * TOKS_PER_CORE]
        cyc = slab.reshape(CYCLES_PER_CORE, CYCLE_TOKS, H_DIM)
        h1 = np.zeros((128, CYCLES_PER_CORE, 3, H_DIM + 2), np.float32)
        for t, (off, L) in enumerate(GROUP_TYPES):
            h1[0:L, :, t, 0:H_DIM] = cyc[:, off : off + L, :].transpose(1, 0, 2)
            h1[0:L, :, t, H_DIM:] = 1.0
        ht = np.zeros((128, TOKS_PER_CORE), np.float16)
        ht[0:H_DIM] = slab.T
        ht[H_DIM : H_DIM + 8] = ind
        in_maps.append({"h": h1.astype(ml_dtypes.bfloat16), "ht": ht})

    res = run_bass_kernel_spmd(nc, in_maps, core_ids=list(range(N_CORES)))
    _CACHE["last_res"] = res
    outs = []
    for c in range(N_CORES):
        full = np.asarray(res.results[c]["out"], dtype=np.float32)
        o = np.empty((CYCLES_PER_CORE, CYCLE_TOKS, H_DIM), np.float32)
        for t, (off, L) in enumerate(GROUP_TYPES):
            o[:, off : off + L, :] = full[0:L, :, t, :].transpose(1, 0, 2)
        outs.append(o.reshape(TOKS_PER_CORE, H_DIM))
    return np.concatenate(outs, axis=0).astype(np.float32)


# revision 65
# speedup vs baseline: 1.0040x; 1.0040x over previous
"""Ragged segment self-attention (AttentionHiddenNet) on 8 Trainium2 cores.

Fixed problem instance: h_states [1, 163840, 64] fp32, 4096 segments whose
lengths cycle through [16, 24, 32, 40, 48, 56, 64, 40] (320 tokens / cycle).
Per segment s: ctx_s = softmax(H_s @ H_s^T, axis=-1) @ H_s.

Sharding: 512 consecutive segments (= 64 cycles = 20480 tokens, contiguous
rows) per core; no cross-core communication.

Per-core algorithm: per 320-token cycle, segments pack into 3 groups
(112/104/104 tokens).  The segment mask folds into the score matmul by
augmenting the hidden dim with 8 one-hot rows of value 30 (one per segment
of the cycle): saug[q,k] = h_q.h_k + 900*same_segment(q,k).  Then
    S~ = Taug^T @ Taug       (PE fp16, K and M zero/spill-padded to 128 so
                              fast-weight-load fires: ~62ns/MM)
    U  = exp(S~ - 1000)      (ACT, one [112,320] op per cycle, bf16 out)
         off-segment exp underflows to exactly 0 = the mask; this also makes
         every padded row/col of U exactly 0, so K/M padding is harmless.
    C|Z = U^T @ [H_g | 1]    (PE bf16, U symmetric, M padded to 128, K=112:
                              ~68ns/MM)
    out = C * (1/Z)          (DVE, batched over 2 cycles: one reciprocal
                              [112,2,3,1] + one broadcast multiply, bf16 out
                              upconverted on host)
Spill/stale regions from the M=128 padding only ever reach output rows
>= L of each group, which the host-side unpack discards.
I/O: host pre-packs token-major tensors so every DMA moves >=6KB per
partition.  ta (fp16 scores operand) on the sync HWDGE ring, h_t (bf16 ctx
operand) on the scalar HWDGE ring, outputs on gpsimd SWDGE (keeps DMA
triggers off the busy ACT queue).  Graduated chunk sizes (6/10/16/16/16
cycles) with 3 rotating ta buffers and 1-2 chunk prefetch distance hide
the transfer ramp.  Steady state is ACT-bound at ~410ns per 320-token
cycle; ~7us NEFF preamble and ~11us semaphore-reset teardown are fixed
framework overhead.
"""

import numpy as np

H_DIM = 64
NUM_SEQS = 4096
LEN_PATTERN = [16, 24, 32, 40, 48, 56, 64, 40]
N_TOTAL = 163840
N_CORES = 8
SEGS_PER_CORE = NUM_SEQS // N_CORES          # 512
CYCLE_TOKS = sum(LEN_PATTERN)                # 320
CYCLES_PER_CORE = SEGS_PER_CORE // len(LEN_PATTERN)   # 64
TOKS_PER_CORE = CYCLES_PER_CORE * CYCLE_TOKS          # 20480

GROUP_TYPES = [(0, 112), (112, 104), (216, 104)]
SEG_STARTS = [0, 16, 40, 72, 112, 160, 216, 280]
IND_VAL = 30.0        # sqrt(900): on-segment score boost +900
BIAS = -1000.0        # exp(s + 900 - 1000) on-seg, exp(s - 1000) -> 0 off-seg

# uneven chunks: small first chunk so compute starts early
CHUNKS = [(0, 6), (6, 10), (16, 16), (32, 16), (48, 16)]  # (start_cyc, n_cyc)
MAX_CYC = 16
N_CHUNKS = len(CHUNKS)

_CACHE = {}


def _expected_sse():
    lens = np.tile(np.array(LEN_PATTERN, dtype=np.int64), NUM_SEQS // len(LEN_PATTERN))
    ends = np.cumsum(lens)
    starts = np.concatenate([[0], ends[:-1]])
    return np.stack([starts, ends], axis=1)


def _build_bass():
    import concourse.bass as bass
    import concourse.bacc as bacc
    import concourse.tile as tile
    from concourse import mybir
    from contextlib import ExitStack

    f32 = mybir.dt.float32
    f16 = mybir.dt.float16
    bf16 = mybir.dt.bfloat16

    nc = bacc.Bacc("TRN2")
    # token-major padded groups: [128, 64 cyc, 3 types, 66]; rows >= L zero
    h_d = nc.dram_tensor(
        "h", [112, CYCLES_PER_CORE, 3, H_DIM + 2], bf16, kind="ExternalInput"
    )
    # augmented transpose, host-built: rows 0-63 h^T, 64-71 segment
    # indicators (value 30 -> +900 on-segment), 72-127 zeros (K=128 pad)
    ht_d = nc.dram_tensor("ht", [128, TOKS_PER_CORE], f16, kind="ExternalInput")
    out_d = nc.dram_tensor(
        "out", [112, CYCLES_PER_CORE, 3, H_DIM], bf16, kind="ExternalOutput"
    )

    with tile.TileContext(nc) as tc, ExitStack() as ctx:
        singles = ctx.enter_context(tc.tile_pool(name="singles", bufs=1))
        iopool = ctx.enter_context(tc.tile_pool(name="iopool", bufs=3))
        upool = ctx.enter_context(tc.tile_pool(name="upool", bufs=4))
        zpool = ctx.enter_context(tc.tile_pool(name="zpool", bufs=4))
        ps_s = ctx.enter_context(tc.tile_pool(name="ps_s", bufs=4, space="PSUM"))
        ps_c = ctx.enter_context(tc.tile_pool(name="ps_c", bufs=3, space="PSUM"))

        bias_t = singles.tile([128, 1], f32, tag="bias")
        nc.gpsimd.memset(bias_t[:, :], BIAS)
        # dummy exp to pull ACT_TABLE_LOAD into the preamble
        warm_t = singles.tile([128, 1], f32, tag="warm")
        nc.scalar.activation(
            warm_t[:, :], bias_t[:, :], mybir.ActivationFunctionType.Exp
        )

        # rotating augmented-transpose tiles [128, MAX_CYC*320]
        ta0 = singles.tile([128, MAX_CYC * CYCLE_TOKS], f16, tag="ta0")
        ta1 = singles.tile([128, MAX_CYC * CYCLE_TOKS], f16, tag="ta1")
        ta2 = singles.tile([128, MAX_CYC * CYCLE_TOKS], f16, tag="ta2")
        ta_bufs = [ta0, ta1, ta2]

        def ta_dma(ck):
            cyc0, n = CHUNKS[ck]
            nc.sync.dma_start(
                ta_bufs[ck % 3][:, 0 : n * CYCLE_TOKS],
                bass.AP(
                    ht_d,
                    cyc0 * CYCLE_TOKS,
                    [[TOKS_PER_CORE, 128], [1, n * CYCLE_TOKS]],
                ),
            )

        def ht_dma(ck):
            cyc0, n = CHUNKS[ck]
            ht_tile = iopool.tile([112, MAX_CYC, 3, H_DIM + 2], bf16, tag="h")
            src = bass.AP(
                h_d,
                cyc0 * 3 * (H_DIM + 2),
                [
                    [CYCLES_PER_CORE * 3 * (H_DIM + 2), 112],
                    [3 * (H_DIM + 2), n],
                    [H_DIM + 2, 3],
                    [1, H_DIM + 2],
                ],
            )
            nc.scalar.dma_start(ht_tile[:, 0:n, :, :], src)
            return ht_tile

        ta_dma(0)
        htq = [ht_dma(0), ht_dma(1)]
        ta_dma(1)
        for ck in range(N_CHUNKS):
            _, n_cyc = CHUNKS[ck]
            chunk_toks = n_cyc * CYCLE_TOKS
            ta = ta_bufs[ck % 3]
            h_t = htq[ck]

            o_all = iopool.tile([112, MAX_CYC, 3, H_DIM], bf16, tag="o")
            for cyc in range(n_cyc):
                if cyc == 2 and ck >= 1 and ck + 1 < N_CHUNKS:
                    ta_dma(ck + 1)
                if cyc == max(0, n_cyc - 8) and ck + 2 < N_CHUNKS:
                    htq.append(ht_dma(ck + 2))
                ktok = cyc * CYCLE_TOKS
                s_ps = ps_s.tile([128, CYCLE_TOKS], f32, tag="s")
                for t, (off, L) in enumerate(GROUP_TYPES):
                    a = ktok + off
                    # M pads to 128 with next-group tokens (their U entries
                    # are exactly 0); the last group of the chunk has no
                    # spill room, so it keeps M=L
                    m = 128 if a + 128 <= chunk_toks else L
                    nc.tensor.matmul(
                        s_ps[0:m, off : off + L],
                        ta[0:128, a : a + m],
                        ta[0:128, a : a + L],
                        start=True,
                        stop=True,
                    )

                u = upool.tile([112, CYCLE_TOKS + 32], bf16, tag="u")
                nc.scalar.activation(
                    u[0:112, 0:CYCLE_TOKS],
                    s_ps[0:112, :],
                    mybir.ActivationFunctionType.Exp,
                    bias=bias_t[0:112, :],
                )

                if cyc % 2 == 0:
                    c_ps = ps_c.tile([128, 2, 3, H_DIM + 2], f32, tag="c")
                for t, (off, L) in enumerate(GROUP_TYPES):
                    # contraction over keys: rows L..112 of u are exact 0
                    # except for the unpadded last group, which must stop at L
                    k = 112 if ktok + off + 128 <= chunk_toks else L
                    nc.tensor.matmul(
                        c_ps[0:128, cyc % 2, t, :],
                        u[0:k, off : off + 128],
                        h_t[0:k, cyc, t, :],
                        start=True,
                        stop=True,
                    )
                if cyc % 2 == 1:
                    r = zpool.tile([112, 2, 3, 1], f32, tag="r")
                    nc.vector.reciprocal(
                        r[0:112, :, :, :], c_ps[0:112, :, :, H_DIM : H_DIM + 1]
                    )
                    nc.vector.tensor_tensor(
                        o_all[0:112, cyc - 1 : cyc + 1, :, :],
                        c_ps[0:112, :, :, 0:H_DIM],
                        r[0:112, :, :, :].to_broadcast((112, 2, 3, H_DIM)),
                        mybir.AluOpType.mult,
                    )

            # two out-DMAs per chunk; the last chunk splits unevenly so the
            # final (teardown-blocking) transfer is small
            splits = (
                [(0, 12), (12, 4)] if ck == N_CHUNKS - 1
                else [(0, n_cyc // 2), (n_cyc // 2, n_cyc // 2)]
            )
            # last chunk's outputs go via the (idle) sync HWDGE ring: its
            # ~0.6us completion beats the ~3us SWDGE end-of-kernel drain
            out_eng = nc.sync if ck == N_CHUNKS - 1 else nc.gpsimd
            for c0, cn in splits:
                dst = bass.AP(
                    out_d,
                    (CHUNKS[ck][0] + c0) * 3 * H_DIM,
                    [
                        [CYCLES_PER_CORE * 3 * H_DIM, 112],
                        [3 * H_DIM, cn],
                        [H_DIM, 3],
                        [1, H_DIM],
                    ],
                )
                out_eng.dma_start(dst, o_all[:, c0 : c0 + cn, :, :])

    nc.compile()
    return nc


def _run_numpy(h, sse):
    # generic host fallback (only used if the input does not match the
    # hardcoded segment pattern)
    out = np.empty_like(h)
    for s, e in sse:
        seg = h[s:e]
        sc = seg @ seg.T
        sc -= sc.max(axis=-1, keepdims=True)
        u = np.exp(sc)
        out[s:e] = (u / u.sum(axis=-1, keepdims=True)) @ seg
    return out


def kernel(h_states, seq_start_end):
    h = np.asarray(h_states, dtype=np.float32).reshape(-1, H_DIM)
    sse = np.asarray(seq_start_end).astype(np.int64)

    if h.shape[0] != N_TOTAL or not np.array_equal(sse, _expected_sse()):
        return _run_numpy(h, sse).astype(np.float32)

    from concourse.bass_utils import run_bass_kernel_spmd
    import ml_dtypes

    if "nc" not in _CACHE:
        _CACHE["nc"] = _build_bass()
    nc = _CACHE["nc"]

    ind = np.zeros((8, CYCLE_TOKS), np.float16)
    for j in range(8):
        ind[j, SEG_STARTS[j] : SEG_STARTS[j] + LEN_PATTERN[j]] = IND_VAL
    ind = np.tile(ind, (1, CYCLES_PER_CORE))

    in_maps = []
    for c in range(N_CORES):
        slab = h[c * TOKS_PER_CORE : (c + 1) * TOKS_PER_CORE]
        cyc = slab.reshape(CYCLES_PER_CORE, CYCLE_TOKS, H_DIM)
        h1 = np.zeros((112, CYCLES_PER_CORE, 3, H_DIM + 2), np.float32)
        for t, (off, L) in enumerate(GROUP_TYPES):
            h1[0:L, :, t, 0:H_DIM] = cyc[:, off : off + L, :].transpose(1, 0, 2)
            h1[0:L, :, t, H_DIM:] = 1.0
        ht = np.zeros((128, TOKS_PER_CORE), np.float16)
        ht[0:H_DIM] = slab.T
        ht[H_DIM : H_DIM + 8] = ind
        in_maps.append({"h": h1.astype(ml_dtypes.bfloat16), "ht": ht})

    res = run_bass_kernel_spmd(nc, in_maps, core_ids=list(range(N_CORES)))
    _CACHE["last_res"] = res
    outs = []
    for c in range(N_CORES):
        full = np.asarray(res.results[c]["out"], dtype=np.float32)
        o = np.empty((CYCLES_PER_CORE, CYCLE_TOKS, H_DIM), np.float32)
        for t, (off, L) in enumerate(GROUP_TYPES):
            o[:, off : off + L, :] = full[0:L, :, t, :].transpose(1, 0, 2)
        outs.append(o.reshape(TOKS_PER_CORE, H_DIM))
    return np.concatenate(outs, axis=0).astype(np.float32)
